# revision 7
# baseline (speedup 1.0000x reference)
"""Trainium2 Bass kernel for JacobianRegulariser2D.

reference math (f32, H=W=4096):
  dy = central diff along H, dx = central diff along W (3-tap [0.5,0,-0.5], zero pad)
  crop [2:-2, 2:-2] -> 4092x4092 (zero padding never reaches the crop)
  j00 = dy[0]+1; j01 = dy[1]; j10 = dx[0]; j11 = dx[1]+1
  det = j00*j11 - j10*j01
  out = mean(relu(-det)^2)

With A = ux[i-1]-ux[i+1], B = uy[i-1]-uy[i+1], C = ux[j-1]-ux[j+1],
D = uy[j-1]-uy[j+1]:  det = 0.25*((A+2)(D+2) - C*B)
  relu(-det)^2 = (1/16) * max((C*B - (A+2)(D+2)), 0)^2

Sharding: H split across 8 cores (512 out rows each, last core's tail rows
masked), halos materialised host-side by overlapping the input slices.
Per core: 4 row-tiles of 128 out rows; row diffs (A,B) on the PE via a
banded +-1 stencil matmul plus a 2-row halo matmul; col diffs (C,D) on the
DVE via shifted free-dim slices in bf16; (A+2)/B leave PSUM through ACT
copies; relu^2-sum fused into an ACT Square with per-partition row mask and
accum_out. Final reduction of the per-core [128, nchunk] partials on host.
"""

import sys

import numpy as np

sys.path.insert(0, "/opt/trn_rl_repo")

import concourse.tile as tile  # noqa: E402
from concourse import bacc, mybir  # noqa: E402
from concourse.bass_utils import run_bass_kernel_spmd  # noqa: E402

P = 128
H = 4096
W = 4096
N_CORES = 8
ROWS_PER_CORE = 512          # out rows computed per core (some masked on core 7)
N_TILES = 4                  # 4 x 128 = 512
STRIP_ROWS = ROWS_PER_CORE + 2
OUT_R0, OUT_R1 = 2, 4094     # valid out rows [2, 4093]
OUT_C0 = 2                   # valid out cols [2, 4093]
OUT_COLS = 4092
COL_CHUNKS = [(2, 2048), (2050, 2044)]  # (c0, n) covering cols 2..4093
NCHUNK = N_TILES * len(COL_CHUNKS)

F32 = mybir.dt.float32
BF16 = mybir.dt.bfloat16

_BF16_NP = mybir.dt.np(BF16)


def _stencil_weights():
    """lhsT [128,128]: out[i] = in[i-1] - in[i+1]  (lhsT[k,i]: +1 at k=i-1, -1 at k=i+1)."""
    w = np.zeros((P, P), dtype=np.float32)
    idx = np.arange(P - 1)
    w[idx, idx + 1] = 1.0    # k = i-1 -> +1
    w[idx + 1, idx] = -1.0   # k = i+1 -> -1
    return w.astype(_BF16_NP)


def _halo_weights():
    """lhsT [2,128]: halo row0 (strip row above tile) feeds out[0] with +1;
    halo row1 (strip row below tile) feeds out[127] with -1."""
    w = np.zeros((2, P), dtype=np.float32)
    w[0, 0] = 1.0
    w[1, P - 1] = -1.0
    return w.astype(_BF16_NP)


def _build_program():
    nc = bacc.Bacc("TRN2", target_bir_lowering=False)

    ux = nc.dram_tensor("ux", [STRIP_ROWS, W], F32, kind="ExternalInput")
    uy = nc.dram_tensor("uy", [STRIP_ROWS, W], F32, kind="ExternalInput")
    wst = nc.dram_tensor("wst", [P, P], BF16, kind="ExternalInput")
    whalo = nc.dram_tensor("whalo", [2, P], BF16, kind="ExternalInput")
    maskd = nc.dram_tensor("mask", [P, N_TILES], F32, kind="ExternalInput")
    outd = nc.dram_tensor("out", [P, NCHUNK], F32, kind="ExternalOutput")

    with tile.TileContext(nc) as tc:
        with (
            tc.tile_pool(name="const", bufs=1) as const_pool,
            tc.tile_pool(name="inp", bufs=3) as inp_pool,
            tc.tile_pool(name="halo", bufs=3) as halo_pool,
            tc.tile_pool(name="work", bufs=2) as work_pool,
            tc.tile_pool(name="acc", bufs=1) as acc_pool,
            tc.tile_pool(name="psum", bufs=1, space="PSUM") as psum_pool,
        ):
            wst_sb = const_pool.tile([P, P], BF16)
            nc.sync.dma_start(out=wst_sb, in_=wst[:, :])
            whalo_sb = const_pool.tile([2, P], BF16)
            nc.sync.dma_start(out=whalo_sb, in_=whalo[:, :])
            mask_sb = const_pool.tile([P, N_TILES], F32)
            nc.sync.dma_start(out=mask_sb, in_=maskd[:, :])

            racc = acc_pool.tile([P, NCHUNK], F32)

            for t in range(N_TILES):
                # strip rows: main tile = [128t+1, 128t+129); halo rows 128t, 128t+129
                r_main = 128 * t + 1
                for ci, (c0, n) in enumerate(COL_CHUNKS):
                    idx = t * len(COL_CHUNKS) + ci
                    ux_t = inp_pool.tile([P, n + 2], BF16, tag="ux_t")
                    uy_t = inp_pool.tile([P, n + 2], BF16, tag="uy_t")
                    # f32 -> bf16 cast during SWDGE DMA
                    nc.gpsimd.dma_start(
                        out=ux_t, in_=ux[r_main : r_main + P, c0 - 1 : c0 + n + 1]
                    )
                    nc.gpsimd.dma_start(
                        out=uy_t, in_=uy[r_main : r_main + P, c0 - 1 : c0 + n + 1]
                    )
                    uxh = halo_pool.tile([2, n], BF16, tag="uxh")
                    uyh = halo_pool.tile([2, n], BF16, tag="uyh")
                    # rows r_main-1 and r_main+128 (stride 129)
                    nc.gpsimd.dma_start(
                        out=uxh,
                        in_=ux[r_main - 1 : r_main + P + 1 : P + 1, c0 : c0 + n],
                    )
                    nc.gpsimd.dma_start(
                        out=uyh,
                        in_=uy[r_main - 1 : r_main + P + 1 : P + 1, c0 : c0 + n],
                    )

                    a_ps = psum_pool.tile([P, n], F32, tag="a_ps")
                    b_ps = psum_pool.tile([P, n], F32, tag="b_ps")
                    for j in range(0, n, 512):
                        e = min(j + 512, n)
                        nc.tensor.matmul(
                            a_ps[:, j:e], wst_sb, ux_t[:, 1 + j : 1 + e],
                            start=True, stop=False,
                        )
                        nc.tensor.matmul(
                            a_ps[:, j:e], whalo_sb, uxh[:, j:e],
                            start=False, stop=True,
                        )
                    for j in range(0, n, 512):
                        e = min(j + 512, n)
                        nc.tensor.matmul(
                            b_ps[:, j:e], wst_sb, uy_t[:, 1 + j : 1 + e],
                            start=True, stop=False,
                        )
                        nc.tensor.matmul(
                            b_ps[:, j:e], whalo_sb, uyh[:, j:e],
                            start=False, stop=True,
                        )

                    # ACT: A2 = A + 2 (PSUM->SBUF, bf16), Bsb = B (PSUM->SBUF, bf16)
                    a2 = work_pool.tile([P, n], BF16, tag="a2")
                    nc.scalar.activation(
                        a2, a_ps, mybir.ActivationFunctionType.Copy, bias=2.0
                    )
                    bsb = work_pool.tile([P, n], BF16, tag="bsb")
                    nc.scalar.activation(
                        bsb, b_ps, mybir.ActivationFunctionType.Copy
                    )

                    # DVE (bf16 2x): col diffs + products
                    c_ = work_pool.tile([P, n], BF16, tag="c_")
                    nc.vector.tensor_sub(c_, ux_t[:, 0:n], ux_t[:, 2 : n + 2])
                    d_ = work_pool.tile([P, n], BF16, tag="d_")
                    nc.vector.tensor_sub(d_, uy_t[:, 0:n], uy_t[:, 2 : n + 2])
                    d2 = work_pool.tile([P, n], BF16, tag="d2")
                    nc.vector.tensor_scalar_add(d2, d_, 2.0)
                    m1 = work_pool.tile([P, n], BF16, tag="m1")
                    nc.vector.tensor_mul(m1, a2, d2)
                    q = work_pool.tile([P, n], BF16, tag="q")
                    nc.vector.tensor_mul(q, c_, bsb)
                    s = work_pool.tile([P, n], BF16, tag="s")
                    nc.vector.tensor_sub(s, q, m1)
                    m = work_pool.tile([P, n], BF16, tag="m")
                    nc.vector.tensor_scalar_max(m, s, 0.0)

                    # ACT: r = Square(m * mask); racc[:, idx] = sum_j r
                    rdump = work_pool.tile([P, n], BF16, tag="rdump")
                    nc.scalar.activation(
                        rdump,
                        m,
                        mybir.ActivationFunctionType.Square,
                        scale=mask_sb[:, t : t + 1],
                        accum_out=racc[:, idx : idx + 1],
                    )

            nc.sync.dma_start(out=outd[:, :], in_=racc)

    nc.compile()
    return nc


_NC_CACHE = None


def _get_program():
    global _NC_CACHE
    if _NC_CACHE is None:
        _NC_CACHE = _build_program()
    return _NC_CACHE


def _make_in_maps(displacement: np.ndarray):
    disp = np.asarray(displacement, dtype=np.float32)
    ux = disp[0, 0]
    uy = disp[0, 1]

    wst = _stencil_weights()
    whalo = _halo_weights()

    in_maps = []
    for k in range(N_CORES):
        g0 = OUT_R0 + ROWS_PER_CORE * k          # first out row of this core
        r0 = g0 - 1                              # first strip row
        r1 = r0 + STRIP_ROWS                     # may exceed H on core 7
        if r1 <= H:
            ux_s = ux[r0:r1]
            uy_s = uy[r0:r1]
        else:
            pad = r1 - H
            ux_s = np.concatenate([ux[r0:H], np.repeat(ux[H - 1 :], pad, 0)], axis=0)
            uy_s = np.concatenate([uy[r0:H], np.repeat(uy[H - 1 :], pad, 0)], axis=0)
        # mask[p, t] = 1 if out row g0 + 128 t + p is a valid out row
        tt, pp = np.meshgrid(np.arange(N_TILES), np.arange(P), indexing="xy")
        rows = g0 + 128 * tt + pp                # [P, N_TILES]
        mask = ((rows >= OUT_R0) & (rows < OUT_R1)).astype(np.float32)
        in_maps.append(
            {
                "ux": np.ascontiguousarray(ux_s),
                "uy": np.ascontiguousarray(uy_s),
                "wst": wst,
                "whalo": whalo,
                "mask": np.ascontiguousarray(mask),
            }
        )
    return in_maps


def kernel(displacement: np.ndarray) -> np.ndarray:
    in_maps = _make_in_maps(displacement)
    nc = _get_program()
    res = run_bass_kernel_spmd(nc, in_maps, core_ids=list(range(N_CORES)))
    total = np.float64(0.0)
    for k in range(N_CORES):
        total += np.asarray(res.results[k]["out"], dtype=np.float64).sum()
    mean = total / (16.0 * OUT_COLS * OUT_COLS)
    return np.float32(mean)
